# revision 4
# baseline (speedup 1.0000x reference)
"""3D depth_to_space (block=2, channels_last) Trainium2 Bass kernel.

Full input  (4, 32, 64, 64, 128) f32  ->  full output (4, 64, 128, 128, 16) f32
    out[n, 2z+dz, 2y+dy, 2x+dx, co] = in[n, z, y, x, dz*64 + dy*32 + dx*16 + co]

Sharding: data-parallel over (batch, D-half). Core c handles n = c//2 and
z in [16*(c%2), 16*(c%2)+16) — a contiguous 32 MiB input slab producing a
contiguous 32 MiB output slab. No collectives.

Per-core program (8 iterations, one z-pair each, double-buffered, raw bass):
  ACT  : load  x[j] HBM -> SBUF tin[j%2]   [128p=(z2,y) x 8192 f32] (32 KB/part runs)
  DVE  : shuffle     tin -> tout           (x, dz, dy, u) -> (dz, dy, x, u) per partition
  SP   : store tout halves -> HBM          2 DMAs (z2 = partitions 0-63 / 64-127,
                                           disjoint SBUF port sets), 16 KB HBM runs
Loads ride the ACT HWDGE ring, stores the SP ring -> they overlap; each
instruction carries <=2 semaphore waits (sequencer limit is ~3).
"""

import numpy as np

import concourse.bass as bass
import concourse.mybir as mybir
from concourse.bass_utils import run_bass_kernel_spmd

B, D, H, W, C = 4, 32, 64, 64, 128
N_CORES = 8
Z_PER_CORE = D // 2        # 16
N_PAIR = Z_PER_CORE // 2   # 8 z-pair iterations per core
F32 = mybir.dt.float32

_NC = None


def _build_nc() -> bass.Bass:
    nc = bass.Bass()
    # x: per-core shard viewed as [z-pair, (z2,y), x*128+c]
    x = nc.declare_dram_parameter("x", [N_PAIR, 128, 8192], F32, isOutput=False)
    # y: per-core output viewed as [z-pair, zo_local(4), yo(128), xo*16+co(2048)]
    y = nc.declare_dram_parameter("y", [N_PAIR, 4, 128, 2048], F32, isOutput=True)

    # Completion order across distinct in-flight DMAs is NOT guaranteed, so
    # each semaphore below has exactly one ordered chain of incrementers:
    #   L[s]      — load into tin slot s        (one in flight per slot)
    #   S[s][z2]  — store of tout slot s, half z2
    #   C         — DVE copy count (single engine, ordered via drain)
    with (
        nc.sbuf_tensor([128, 2 * 8192], F32) as tin,   # 2 slots
        nc.sbuf_tensor([128, 2 * 8192], F32) as tout,  # 2 slots
        nc.semaphore("sem_l0") as l0,
        nc.semaphore("sem_l1") as l1,
        nc.semaphore("sem_c") as sem_c,
        nc.semaphore("sem_s00") as s00,
        nc.semaphore("sem_s01") as s01,
        nc.semaphore("sem_s10") as s10,
        nc.semaphore("sem_s11") as s11,
        nc.Block() as block,
    ):
        L = [l0, l1]
        S = [[s00, s01], [s10, s11]]

        @block.scalar
        def _(act):
            for j in range(N_PAIR):
                if j >= 2:
                    # tin slot j%2 is read by copy(j-2)
                    act.wait_ge(sem_c, j - 1)
                act.dma_start(
                    out=tin[:, (j % 2) * 8192 : (j % 2 + 1) * 8192], in_=x[j]
                ).then_inc(L[j % 2], 16)

        @block.vector
        def _(vector):
            for j in range(N_PAIR):
                s = j % 2
                vector.wait_ge(L[s], 16 * (j // 2 + 1))
                if j >= 2:
                    # tout slot s is read by the 2 stores of iteration j-2
                    vector.wait_ge(S[s][0], 16 * (j // 2))
                    vector.wait_ge(S[s][1], 16 * (j // 2))
                off = s * 8192
                inv = tin[:, off : off + 8192].rearrange(
                    "p (x dz dy u) -> p dz dy x u", x=64, dz=2, dy=2, u=32
                )
                outv = tout[:, off : off + 8192].rearrange(
                    "p (dz dy x u) -> p dz dy x u", dz=2, dy=2, x=64, u=32
                )
                vector.tensor_copy(out=outv, in_=inv)
                # DVE sem updates must ride a DRAIN: a raw inc on the copy can
                # fire while reads/writes are still in the DVE pipeline.
                vector.drain().then_inc(sem_c, 1)

        @block.sync
        def _(sync):
            for j in range(N_PAIR):
                s = j % 2
                sync.wait_ge(sem_c, j + 1)
                off = s * 8192
                for z2 in range(2):
                    srcv = tout[z2 * 64 : (z2 + 1) * 64, off : off + 8192].rearrange(
                        "yy (dz dy e) -> yy dz dy e", dz=2, dy=2, e=2048
                    )
                    dstv = y[j, 2 * z2 : 2 * z2 + 2].rearrange(
                        "dz (yy dy) e -> yy dz dy e", yy=64, dy=2
                    )
                    sync.dma_start(out=dstv, in_=srcv).then_inc(S[s][z2], 16)
            for s in range(2):
                for z2 in range(2):
                    sync.wait_ge(S[s][z2], 16 * (N_PAIR // 2))

    return nc


def _get_nc() -> bass.Bass:
    global _NC
    if _NC is None:
        _NC = _build_nc()
    return _NC


def _shard(x: np.ndarray, c: int) -> np.ndarray:
    n, zh = c // 2, c % 2
    s = np.ascontiguousarray(x[n, zh * Z_PER_CORE : (zh + 1) * Z_PER_CORE])
    return s.reshape(N_PAIR, 128, 8192)


def run(inputs: np.ndarray, trace: bool = False):
    x = np.ascontiguousarray(np.asarray(inputs, dtype=np.float32))
    assert x.shape == (B, D, H, W, C), x.shape
    in_maps = [{"x": _shard(x, c)} for c in range(N_CORES)]
    res = run_bass_kernel_spmd(
        _get_nc(), in_maps, core_ids=list(range(N_CORES)), trace=trace
    )
    out = np.empty((B, 2 * D, 2 * H, 2 * W, C // 8), np.float32)
    for c in range(N_CORES):
        n, zh = c // 2, c % 2
        out[n, zh * 2 * Z_PER_CORE : (zh + 1) * 2 * Z_PER_CORE] = res.results[c][
            "y"
        ].reshape(2 * Z_PER_CORE, 2 * H, 2 * W, C // 8)
    return out, res


def kernel(**inputs) -> np.ndarray:
    out, _ = run(inputs["inputs"], trace=False)
    return out


# revision 6
# speedup vs baseline: 2.3789x; 2.3789x over previous
"""3D depth_to_space (block=2, channels_last) Trainium2 Bass kernel.

Full input  (4, 32, 64, 64, 128) f32  ->  full output (4, 64, 128, 128, 16) f32
    out[n, 2z+dz, 2y+dy, 2x+dx, co] = in[n, z, y, x, dz*64 + dy*32 + dx*16 + co]

Sharding: data-parallel over (batch, D-half). Core c handles n = c//2 and
z in [16*(c%2), 16*(c%2)+16) — a contiguous 32 MiB input slab producing a
contiguous 32 MiB output slab. No collectives.

Per-core program (8 iterations, one z-pair each, double-buffered, raw bass):
  ACT  : load  x[j] HBM -> SBUF tin[j%2]   [128p=(z2,y) x 8192 f32] (32 KB/part runs)
  DVE  : shuffle     tin -> tout           (x, dz, dy, u) -> (dz, dy, x, u) per partition
  SP   : store tout halves -> HBM          2 DMAs (z2 = partitions 0-63 / 64-127,
                                           disjoint SBUF port sets), 16 KB HBM runs
Loads ride the ACT HWDGE ring, stores the SP ring -> they overlap; each
instruction carries <=2 semaphore waits (sequencer limit is ~3).
"""

import numpy as np

import concourse.bass as bass
import concourse.mybir as mybir
from concourse.bass_utils import run_bass_kernel_spmd

B, D, H, W, C = 4, 32, 64, 64, 128
N_CORES = 8
Z_PER_CORE = D // 2        # 16
N_PAIR = Z_PER_CORE // 2   # 8 z-pair iterations per core
F32 = mybir.dt.float32

_NC = None


def _build_nc(repeats: int = 1) -> bass.Bass:
    # repeats > 1 re-runs the whole pipeline on the same data inside one NEFF
    # (benchmarking only — lets device time dominate dispatch noise).
    n_iter = N_PAIR * repeats
    nc = bass.Bass()
    # x: per-core shard viewed as [z-pair, (z2,y), x*128+c]
    x = nc.declare_dram_parameter("x", [N_PAIR, 128, 8192], F32, isOutput=False)
    # y: per-core output viewed as [z-pair, zo_local(4), yo(128), xo*16+co(2048)]
    y = nc.declare_dram_parameter("y", [N_PAIR, 4, 128, 2048], F32, isOutput=True)

    # Completion order across distinct in-flight DMAs is NOT guaranteed, so
    # each semaphore below has exactly one ordered chain of incrementers:
    #   L[s]      — load into tin slot s        (one in flight per slot)
    #   S[s][z2]  — store of tout slot s, half z2
    #   C         — DVE copy count (single engine, ordered via drain)
    with (
        nc.sbuf_tensor([128, 2 * 8192], F32) as tin,   # 2 slots
        nc.sbuf_tensor([128, 2 * 8192], F32) as tout,  # 2 slots
        nc.semaphore("sem_l0") as l0,
        nc.semaphore("sem_l1") as l1,
        nc.semaphore("sem_c") as sem_c,
        nc.semaphore("sem_s00") as s00,
        nc.semaphore("sem_s01") as s01,
        nc.semaphore("sem_s10") as s10,
        nc.semaphore("sem_s11") as s11,
        nc.Block() as block,
    ):
        L = [l0, l1]
        S = [[s00, s01], [s10, s11]]

        @block.scalar
        def _(act):
            for j in range(n_iter):
                if j >= 2:
                    # tin slot j%2 is read by copy(j-2)
                    act.wait_ge(sem_c, j - 1)
                act.dma_start(
                    out=tin[:, (j % 2) * 8192 : (j % 2 + 1) * 8192], in_=x[j % N_PAIR]
                ).then_inc(L[j % 2], 16)

        @block.vector
        def _(vector):
            for j in range(n_iter):
                s = j % 2
                vector.wait_ge(L[s], 16 * (j // 2 + 1))
                if j >= 2:
                    # tout slot s is read by the 2 stores of iteration j-2
                    vector.wait_ge(S[s][0], 16 * (j // 2))
                    vector.wait_ge(S[s][1], 16 * (j // 2))
                off = s * 8192
                inv = tin[:, off : off + 8192].rearrange(
                    "p (x dz dy u) -> p dz dy x u", x=64, dz=2, dy=2, u=32
                )
                outv = tout[:, off : off + 8192].rearrange(
                    "p (dz dy x u) -> p dz dy x u", dz=2, dy=2, x=64, u=32
                )
                vector.tensor_copy(out=outv, in_=inv)
                # DVE sem updates must ride a DRAIN: a raw inc on the copy can
                # fire while reads/writes are still in the DVE pipeline.
                vector.drain().then_inc(sem_c, 1)

        @block.sync
        def _(sync):
            for j in range(n_iter):
                s = j % 2
                sync.wait_ge(sem_c, j + 1)
                off = s * 8192
                for z2 in range(2):
                    srcv = tout[z2 * 64 : (z2 + 1) * 64, off : off + 8192].rearrange(
                        "yy (dz dy e) -> yy dz dy e", dz=2, dy=2, e=2048
                    )
                    dstv = y[j % N_PAIR, 2 * z2 : 2 * z2 + 2].rearrange(
                        "dz (yy dy) e -> yy dz dy e", yy=64, dy=2
                    )
                    sync.dma_start(out=dstv, in_=srcv).then_inc(S[s][z2], 16)
            for s in range(2):
                for z2 in range(2):
                    sync.wait_ge(S[s][z2], 16 * (n_iter // 2))

    return nc


def _get_nc() -> bass.Bass:
    global _NC
    if _NC is None:
        _NC = _build_nc()
    return _NC


def _shard(x: np.ndarray, c: int) -> np.ndarray:
    n, zh = c // 2, c % 2
    s = np.ascontiguousarray(x[n, zh * Z_PER_CORE : (zh + 1) * Z_PER_CORE])
    return s.reshape(N_PAIR, 128, 8192)


def run(inputs: np.ndarray, trace: bool = False):
    x = np.ascontiguousarray(np.asarray(inputs, dtype=np.float32))
    assert x.shape == (B, D, H, W, C), x.shape
    in_maps = [{"x": _shard(x, c)} for c in range(N_CORES)]
    res = run_bass_kernel_spmd(
        _get_nc(), in_maps, core_ids=list(range(N_CORES)), trace=trace
    )
    out = np.empty((B, 2 * D, 2 * H, 2 * W, C // 8), np.float32)
    for c in range(N_CORES):
        n, zh = c // 2, c % 2
        out[n, zh * 2 * Z_PER_CORE : (zh + 1) * 2 * Z_PER_CORE] = res.results[c][
            "y"
        ].reshape(2 * Z_PER_CORE, 2 * H, 2 * W, C // 8)
    return out, res


def kernel(**inputs) -> np.ndarray:
    out, _ = run(inputs["inputs"], trace=False)
    return out


# revision 8
# speedup vs baseline: 2.3820x; 1.0013x over previous
"""3D depth_to_space (block=2, channels_last) Trainium2 Bass kernel.

Full input  (4, 32, 64, 64, 128) f32  ->  full output (4, 64, 128, 128, 16) f32
    out[n, 2z+dz, 2y+dy, 2x+dx, co] = in[n, z, y, x, dz*64 + dy*32 + dx*16 + co]

Sharding: data-parallel over (batch, D-half). Core c handles n = c//2 and
z in [16*(c%2), 16*(c%2)+16) — a contiguous 32 MiB input slab producing a
contiguous 32 MiB output slab. No collectives.

Per-core program (8 iterations, one z-pair each, double-buffered, raw bass):
  ACT  : load  x[j] HBM -> SBUF tin[j%2]   [128p=(z2,y) x 8192 f32] (32 KB/part runs)
  DVE  : shuffle     tin -> tout           (x, dz, dy, u) -> (dz, dy, x, u) per partition
  SP   : store tout halves -> HBM          2 DMAs (z2 = partitions 0-63 / 64-127,
                                           disjoint SBUF port sets), 16 KB HBM runs
Loads ride the ACT HWDGE ring, stores the SP ring -> they overlap; each
instruction carries <=2 semaphore waits (sequencer limit is ~3).
"""

import numpy as np

import concourse.bass as bass
import concourse.mybir as mybir
from concourse.bass_utils import run_bass_kernel_spmd

B, D, H, W, C = 4, 32, 64, 64, 128
N_CORES = 8
Z_PER_CORE = D // 2        # 16
N_PAIR = Z_PER_CORE // 2   # 8 z-pair iterations per core
F32 = mybir.dt.float32

_NC = None


def _build_nc(repeats: int = 1) -> bass.Bass:
    # repeats > 1 re-runs the whole pipeline on the same data inside one NEFF
    # (benchmarking only — lets device time dominate dispatch noise).
    n_iter = N_PAIR * repeats
    nc = bass.Bass()
    # x: per-core shard viewed as [z-pair, (z2,y), x*128+c]
    x = nc.declare_dram_parameter("x", [N_PAIR, 128, 8192], F32, isOutput=False)
    # y: per-core output viewed as [z-pair, zo_local(4), yo(128), xo*16+co(2048)]
    y = nc.declare_dram_parameter("y", [N_PAIR, 4, 128, 2048], F32, isOutput=True)

    # Completion order across distinct in-flight DMAs is NOT guaranteed, so
    # each semaphore below has exactly one ordered chain of incrementers:
    #   L[s]      — load into tin slot s        (one in flight per slot)
    #   S[s][z2]  — store of tout slot s, half z2
    #   C         — DVE copy count (single engine, ordered via drain)
    with (
        nc.sbuf_tensor([128, 2 * 8192], F32) as tin,   # 2 slots
        nc.sbuf_tensor([128, 2 * 8192], F32) as tout,  # 2 slots
        nc.semaphore("sem_l0") as l0,
        nc.semaphore("sem_l1") as l1,
        nc.semaphore("sem_c") as sem_c,
        nc.semaphore("sem_s00") as s00,
        nc.semaphore("sem_s01") as s01,
        nc.semaphore("sem_s10") as s10,
        nc.semaphore("sem_s11") as s11,
        nc.Block() as block,
    ):
        L = [l0, l1]
        S = [[s00, s01], [s10, s11]]

        @block.scalar
        def _(act):
            for j in range(n_iter):
                if j >= 2:
                    # tin slot j%2 is read by copy(j-2)
                    act.wait_ge(sem_c, j - 1)
                act.dma_start(
                    out=tin[:, (j % 2) * 8192 : (j % 2 + 1) * 8192], in_=x[j % N_PAIR]
                ).then_inc(L[j % 2], 16)

        @block.vector
        def _(vector):
            for j in range(n_iter):
                s = j % 2
                vector.wait_ge(L[s], 16 * (j // 2 + 1))
                if j >= 2:
                    # tout slot s is read by the 2 stores of iteration j-2
                    vector.wait_ge(S[s][0], 16 * (j // 2))
                    vector.wait_ge(S[s][1], 16 * (j // 2))
                off = s * 8192
                inv = tin[:, off : off + 8192].rearrange(
                    "p (x dz dy u) -> p dz dy x u", x=64, dz=2, dy=2, u=32
                )
                outv = tout[:, off : off + 8192].rearrange(
                    "p (dz dy x u) -> p dz dy x u", dz=2, dy=2, x=64, u=32
                )
                vector.tensor_copy(out=outv, in_=inv)
                # DVE sem updates must ride a DRAIN: a raw inc on the copy can
                # fire while reads/writes are still in the DVE pipeline.
                vector.drain().then_inc(sem_c, 1)

        @block.sync
        def _(sync):
            for j in range(n_iter):
                s = j % 2
                sync.wait_ge(sem_c, j + 1)
                off = s * 8192
                for z2 in range(2):
                    srcv = tout[z2 * 64 : (z2 + 1) * 64, off : off + 8192].rearrange(
                        "yy (dz dy e) -> yy dz dy e", dz=2, dy=2, e=2048
                    )
                    dstv = y[j % N_PAIR, 2 * z2 : 2 * z2 + 2].rearrange(
                        "dz (yy dy) e -> yy dz dy e", yy=64, dy=2
                    )
                    sync.dma_start(out=dstv, in_=srcv).then_inc(S[s][z2], 16)
            for s in range(2):
                for z2 in range(2):
                    sync.wait_ge(S[s][z2], 16 * (n_iter // 2))

    return nc


def _get_nc() -> bass.Bass:
    global _NC
    if _NC is None:
        _NC = _build_nc()
    return _NC


def _shard(x: np.ndarray, c: int) -> np.ndarray:
    n, zh = c // 2, c % 2
    s = np.ascontiguousarray(x[n, zh * Z_PER_CORE : (zh + 1) * Z_PER_CORE])
    return s.reshape(N_PAIR, 128, 8192)


def _gather(per_core_y) -> np.ndarray:
    out = np.empty((B, 2 * D, 2 * H, 2 * W, C // 8), np.float32)
    for c in range(N_CORES):
        n, zh = c // 2, c % 2
        out[n, zh * 2 * Z_PER_CORE : (zh + 1) * 2 * Z_PER_CORE] = per_core_y[
            c
        ].reshape(2 * Z_PER_CORE, 2 * H, 2 * W, C // 8)
    return out


_EXEC = None  # cached (fn, sharding, zeros) for repeat calls


def _get_exec():
    """Build the jitted shard_map executable once and reuse it — the stock
    run_bass_kernel_spmd path re-lowers + re-jits on every call (~10 s)."""
    global _EXEC
    if _EXEC is not None:
        return _EXEC
    import jax
    from jax.sharding import Mesh, PartitionSpec, NamedSharding
    from jax.experimental.shard_map import shard_map
    from concourse.bass2jax import (
        _bass_exec_p,
        install_neuronx_cc_hook,
        partition_id_tensor,
    )

    install_neuronx_cc_hook()
    nc = _get_nc()
    partition_name = nc.partition_id_tensor.name if nc.partition_id_tensor else None
    out_aval = jax.core.ShapedArray((N_PAIR, 4, 128, 2048), np.float32)
    all_names = ["x", "y"] + ([partition_name] if partition_name else [])

    def _body(xs, ys):
        operands = [xs, ys]
        if partition_name is not None:
            operands.append(partition_id_tensor())
        return _bass_exec_p.bind(
            *operands,
            out_avals=(out_aval,),
            in_names=tuple(all_names),
            out_names=("y",),
            lowering_input_output_aliases=(),
            sim_require_finite=True,
            sim_require_nnan=True,
            nc=nc,
        )[0]

    devices = jax.devices()[:N_CORES]
    mesh = Mesh(np.asarray(devices), ("core",))
    fn = jax.jit(
        shard_map(
            _body,
            mesh=mesh,
            in_specs=(PartitionSpec("core"),) * 2,
            out_specs=PartitionSpec("core"),
            check_rep=False,
        ),
        keep_unused=True,
    )
    sharding = NamedSharding(mesh, PartitionSpec("core"))
    zeros = jax.device_put(
        np.zeros((N_CORES * N_PAIR, 4, 128, 2048), np.float32), sharding
    )
    _EXEC = (fn, sharding, zeros)
    return _EXEC


def run(inputs: np.ndarray, trace: bool = False):
    x = np.ascontiguousarray(np.asarray(inputs, dtype=np.float32))
    assert x.shape == (B, D, H, W, C), x.shape
    shards = [_shard(x, c) for c in range(N_CORES)]
    try:
        import jax

        fn, sharding, zeros = _get_exec()
        concat_in = jax.device_put(np.concatenate(shards, axis=0), sharding)
        out_arr = np.asarray(fn(concat_in, zeros))
        per_core = [out_arr[c * N_PAIR : (c + 1) * N_PAIR] for c in range(N_CORES)]
        return _gather(per_core), None
    except Exception as e:
        # Fallback: stock SPMD runner (slower per call, same NEFF).
        import sys as _sys

        print(f"kernel: cached-exec path failed ({e!r}); "
              "falling back to run_bass_kernel_spmd", file=_sys.stderr)
        in_maps = [{"x": s} for s in shards]
        res = run_bass_kernel_spmd(
            _get_nc(), in_maps, core_ids=list(range(N_CORES)), trace=trace
        )
        return _gather([res.results[c]["y"] for c in range(N_CORES)]), res


def kernel(**inputs) -> np.ndarray:
    out, _ = run(inputs["inputs"], trace=False)
    return out


# revision 11
# speedup vs baseline: 2.3934x; 1.0048x over previous
"""3D depth_to_space (block=2, channels_last) Trainium2 Bass kernel.

Full input  (4, 32, 64, 64, 128) f32  ->  full output (4, 64, 128, 128, 16) f32
    out[n, 2z+dz, 2y+dy, 2x+dx, co] = in[n, z, y, x, dz*64 + dy*32 + dx*16 + co]

Sharding: data-parallel over (batch, D-half). Core c handles n = c//2 and
z in [16*(c%2), 16*(c%2)+16) — a contiguous 32 MiB input slab producing a
contiguous 32 MiB output slab. No collectives.

Per-core program (8 iterations, one z-pair each, double-buffered, raw bass):
  GPSIMD: load  x[j] HBM -> SBUF tin[j%2]  [128p=(z2,y) x 8192 f32] (32 KB/part runs)
  DVE   : shuffle     tin -> tout          (x, dz, dy, u) -> (dz, dy, x, u) per partition
  SP/ACT: store tout halves -> HBM         one 2 MiB DMA per half (z2 = partitions
                                           0-63 on SP ring / 64-127 on ACT ring,
                                           disjoint SBUF port sets), 16 KB HBM runs
Loads ride the SWDGE ring and each HWDGE ring carries one store stream ->
three independent DMA streams; each instruction carries <=2 semaphore waits
(sequencer limit is ~3).
"""

import numpy as np

import concourse.bass as bass
import concourse.mybir as mybir
from concourse.bass_utils import run_bass_kernel_spmd

B, D, H, W, C = 4, 32, 64, 64, 128
N_CORES = 8
Z_PER_CORE = D // 2        # 16
N_PAIR = Z_PER_CORE // 2   # 8 z-pair iterations per core
F32 = mybir.dt.float32

_NC = None


def _build_nc(repeats: int = 1) -> bass.Bass:
    # repeats > 1 re-runs the whole pipeline on the same data inside one NEFF
    # (benchmarking only — lets device time dominate dispatch noise).
    n_iter = N_PAIR * repeats
    nc = bass.Bass()
    # x: per-core shard viewed as [z-pair, (z2,y), x*128+c]
    x = nc.declare_dram_parameter("x", [N_PAIR, 128, 8192], F32, isOutput=False)
    # y: per-core output viewed as [z-pair, zo_local(4), yo(128), xo*16+co(2048)]
    y = nc.declare_dram_parameter("y", [N_PAIR, 4, 128, 2048], F32, isOutput=True)

    # Completion order across distinct in-flight DMAs is NOT guaranteed, so
    # each semaphore below has exactly one ordered chain of incrementers:
    #   L[s]      — load into tin slot s        (one in flight per slot)
    #   S[s][z2]  — store of tout slot s, half z2
    #   C         — DVE copy count (single engine, ordered via drain)
    with (
        nc.sbuf_tensor([128, 2 * 8192], F32) as tin,   # 2 slots
        nc.sbuf_tensor([128, 2 * 8192], F32) as tout,  # 2 slots
        nc.semaphore("sem_l0") as l0,
        nc.semaphore("sem_l1") as l1,
        nc.semaphore("sem_c") as sem_c,
        nc.semaphore("sem_s00") as s00,
        nc.semaphore("sem_s01") as s01,
        nc.semaphore("sem_s10") as s10,
        nc.semaphore("sem_s11") as s11,
        nc.Block() as block,
    ):
        L = [l0, l1]
        S = [[s00, s01], [s10, s11]]

        @block.gpsimd
        def _(gp):
            # Loads ride the SWDGE ring so each HWDGE ring carries only one
            # store stream (SP: z2=0, ACT: z2=1) — three independent DMA
            # streams.
            for j in range(n_iter):
                if j >= 2:
                    # tin slot j%2 is read by copy(j-2)
                    gp.wait_ge(sem_c, j - 1)
                gp.dma_start(
                    out=tin[:, (j % 2) * 8192 : (j % 2 + 1) * 8192], in_=x[j % N_PAIR]
                ).then_inc(L[j % 2], 16)

        @block.vector
        def _(vector):
            for j in range(n_iter):
                s = j % 2
                vector.wait_ge(L[s], 16 * (j // 2 + 1))
                if j >= 2:
                    # tout slot s is read by the 2 stores of iteration j-2
                    vector.wait_ge(S[s][0], 16 * (j // 2))
                    vector.wait_ge(S[s][1], 16 * (j // 2))
                off = s * 8192
                inv = tin[:, off : off + 8192].rearrange(
                    "p (x dz dy u) -> p dz dy x u", x=64, dz=2, dy=2, u=32
                )
                outv = tout[:, off : off + 8192].rearrange(
                    "p (dz dy x u) -> p dz dy x u", dz=2, dy=2, x=64, u=32
                )
                vector.tensor_copy(out=outv, in_=inv)
                # DVE sem updates must ride a DRAIN: a raw inc on the copy can
                # fire while reads/writes are still in the DVE pipeline.
                vector.drain().then_inc(sem_c, 1)

        def _store_half(eng, z2):
            for j in range(n_iter):
                s = j % 2
                eng.wait_ge(sem_c, j + 1)
                off = s * 8192
                srcv = tout[z2 * 64 : (z2 + 1) * 64, off : off + 8192].rearrange(
                    "yy (dz dy e) -> yy dz dy e", dz=2, dy=2, e=2048
                )
                dstv = y[j % N_PAIR, 2 * z2 : 2 * z2 + 2].rearrange(
                    "dz (yy dy) e -> yy dz dy e", yy=64, dy=2
                )
                eng.dma_start(out=dstv, in_=srcv).then_inc(S[s][z2], 16)
            for s in range(2):
                eng.wait_ge(S[s][z2], 16 * (n_iter // 2))

        @block.sync
        def _(sync):
            _store_half(sync, 0)

        @block.scalar
        def _(act):
            _store_half(act, 1)

    return nc


def _get_nc() -> bass.Bass:
    global _NC
    if _NC is None:
        _NC = _build_nc()
    return _NC


def _shard(x: np.ndarray, c: int) -> np.ndarray:
    n, zh = c // 2, c % 2
    s = np.ascontiguousarray(x[n, zh * Z_PER_CORE : (zh + 1) * Z_PER_CORE])
    return s.reshape(N_PAIR, 128, 8192)


def _gather(per_core_y) -> np.ndarray:
    out = np.empty((B, 2 * D, 2 * H, 2 * W, C // 8), np.float32)
    for c in range(N_CORES):
        n, zh = c // 2, c % 2
        out[n, zh * 2 * Z_PER_CORE : (zh + 1) * 2 * Z_PER_CORE] = per_core_y[
            c
        ].reshape(2 * Z_PER_CORE, 2 * H, 2 * W, C // 8)
    return out


_EXEC = None  # cached (fn, sharding, zeros) for repeat calls


def _get_exec():
    """Build the jitted shard_map executable once and reuse it — the stock
    run_bass_kernel_spmd path re-lowers + re-jits on every call (~10 s)."""
    global _EXEC
    if _EXEC is not None:
        return _EXEC
    import jax
    from jax.sharding import Mesh, PartitionSpec, NamedSharding
    from jax.experimental.shard_map import shard_map
    from concourse.bass2jax import (
        _bass_exec_p,
        install_neuronx_cc_hook,
        partition_id_tensor,
    )

    install_neuronx_cc_hook()
    nc = _get_nc()
    partition_name = nc.partition_id_tensor.name if nc.partition_id_tensor else None
    out_aval = jax.core.ShapedArray((N_PAIR, 4, 128, 2048), np.float32)
    all_names = ["x", "y"] + ([partition_name] if partition_name else [])

    def _body(xs, ys):
        operands = [xs, ys]
        if partition_name is not None:
            operands.append(partition_id_tensor())
        return _bass_exec_p.bind(
            *operands,
            out_avals=(out_aval,),
            in_names=tuple(all_names),
            out_names=("y",),
            lowering_input_output_aliases=(),
            sim_require_finite=True,
            sim_require_nnan=True,
            nc=nc,
        )[0]

    devices = jax.devices()[:N_CORES]
    mesh = Mesh(np.asarray(devices), ("core",))
    fn = jax.jit(
        shard_map(
            _body,
            mesh=mesh,
            in_specs=(PartitionSpec("core"),) * 2,
            out_specs=PartitionSpec("core"),
            check_rep=False,
        ),
        keep_unused=True,
    )
    sharding = NamedSharding(mesh, PartitionSpec("core"))
    zeros = jax.device_put(
        np.zeros((N_CORES * N_PAIR, 4, 128, 2048), np.float32), sharding
    )
    _EXEC = (fn, sharding, zeros)
    return _EXEC


def run(inputs: np.ndarray, trace: bool = False):
    x = np.ascontiguousarray(np.asarray(inputs, dtype=np.float32))
    assert x.shape == (B, D, H, W, C), x.shape
    shards = [_shard(x, c) for c in range(N_CORES)]
    try:
        import jax

        fn, sharding, zeros = _get_exec()
        concat_in = jax.device_put(np.concatenate(shards, axis=0), sharding)
        out_arr = np.asarray(fn(concat_in, zeros))
        per_core = [out_arr[c * N_PAIR : (c + 1) * N_PAIR] for c in range(N_CORES)]
        return _gather(per_core), None
    except Exception as e:
        # Fallback: stock SPMD runner (slower per call, same NEFF).
        import sys as _sys

        print(f"kernel: cached-exec path failed ({e!r}); "
              "falling back to run_bass_kernel_spmd", file=_sys.stderr)
        in_maps = [{"x": s} for s in shards]
        res = run_bass_kernel_spmd(
            _get_nc(), in_maps, core_ids=list(range(N_CORES)), trace=trace
        )
        return _gather([res.results[c]["y"] for c in range(N_CORES)]), res


def kernel(**inputs) -> np.ndarray:
    out, _ = run(inputs["inputs"], trace=False)
    return out
